# revision 8
# baseline (speedup 1.0000x reference)
"""CRF negative log-likelihood on 8 Trainium2 NeuronCores.

Math: loss = sum_b (logZ_b - gold_b) for a linear-chain CRF with
B=256, S=2048, T=64, contiguous-prefix masks.

Strategy (pure batch data-parallel, 32 sequences per core):
- start/end transition vectors are folded into feats on the host
  (feats2[b,0,:] += start; feats2[b,len_b-1,:] += end), so the whole
  partition function reduces to a masked product of per-step linear
  operators in the exp domain.
- The recursion runs BACKWARD over time via the identity
  v_tau = E @ (d_tau-1 * v_tau-1) + ones * dm_tau, where E=exp(trans)^T
  is (64,64), d=exp(feats2) and dm is the one-hot of each sequence end.
  The "+ones*dm" injection is folded into the matmul by augmenting the
  stationary operand with a row of ones and streaming dm as a 65th
  SBUF partition of the recursion tile: one 65x64x32 matmul plus one
  (64,32) elementwise multiply per step is the whole serial chain.
- Periodic per-batch rescaling (every 8 steps, using a 2-step-stale
  column sum so it stays off the critical path) keeps the exp-domain
  state in fp32 range; log corrections accumulate per batch element.
- gold score: emission and transition gathers are computed with
  one-hot matmuls: e = sum(featsR * OHm), t = sum((trans^T @ OHm) * OHm
  shifted by one step), with the partition-dim reductions done as
  ones-vector matmuls accumulated into PSUM across the whole kernel.
"""

import numpy as np

B, S, T = 256, 2048, 64
NCORES = 8
BL = B // NCORES          # 32 sequences per core
K = 64                    # time steps per chunk
NCH = S // K              # 32 chunks
RESC = 8                  # rescale every RESC steps
STALE = 2                 # colsum measured this many steps early

_PROGRAM = None


def _build_program():
    import concourse.bass as bass
    import concourse.mybir as mybir
    import concourse.tile as tile
    from bass_rust import ScopedClock

    f32 = mybir.dt.float32
    f32r = mybir.dt.float32r
    u8 = mybir.dt.uint8
    AF = mybir.ActivationFunctionType
    AX = mybir.AxisListType
    ALU = mybir.AluOpType

    class SplitDrainTileContext(tile.TileContext):
        # The walrus build here rejects a Drain carrying >2 sync waits
        # ("Too many sync wait commands"); carry them on SP nops instead.
        def _drain_and_barrier(self, tick_clock, wait_clock):
            nc = self.nc
            gvc = tick_clock.global_clock
            cur = ScopedClock()
            for proc in range(len(gvc)):
                t = gvc[proc]
                if t > 0:
                    sc = ScopedClock()
                    sc.require_at_least(None, proc, t)
                    carrier = nc.sync.nop(nofuse=True, hint="split_drain_wait")
                    wait_clock.add_sem_waits(carrier.ins, sc, cur)
                    cur.update_past(sc)
            drain_inst = nc.sync.drain()
            wait_clock.add_sem_waits(
                drain_inst.ins, ScopedClock({None: tick_clock.global_clock}), cur
            )
            nc.all_engine_barrier()
            assert self.sems is not None
            popped = nc._tile_sem_poison_stack.pop()
            assert popped is self._sem_poison
            nc.clear_and_free_semaphores(list(self.sems.allocated().values()))
            nc.all_engine_barrier()

    nc = bass.Bass("TRN2", target_bir_lowering=False, debug=False)

    featsR = nc.dram_tensor("featsR", [T, S, BL], f32, kind="ExternalInput").ap()
    ohmpad = nc.dram_tensor("ohmpad", [T, S + 1, BL], u8, kind="ExternalInput").ap()
    dmrow = nc.dram_tensor("dmrow", [S, BL], f32, kind="ExternalInput").ap()
    transT = nc.dram_tensor("transT", [T, T], f32, kind="ExternalInput").ap()
    trans = nc.dram_tensor("trans", [T, T], f32r, kind="ExternalInput").ap()
    out_parts = nc.dram_tensor("out_parts", [1, BL], f32, kind="ExternalOutput").ap()

    with SplitDrainTileContext(nc) as tc:
        with (
            tc.tile_pool(name="const", bufs=1) as const_pool,
            tc.tile_pool(name="fr", bufs=2) as fr_pool,
            tc.tile_pool(name="dexp", bufs=2) as d_pool,
            tc.tile_pool(name="ohm8", bufs=2) as ohm8_pool,
            tc.tile_pool(name="ohmf", bufs=2) as ohmf_pool,
            tc.tile_pool(name="vt", bufs=2) as v_pool,
            tc.tile_pool(name="prod", bufs=2) as prod_pool,
            tc.tile_pool(name="small", bufs=4) as small_pool,
            tc.tile_pool(name="cps", bufs=2, space="PSUM") as chain_psum,
            tc.tile_pool(name="gps", bufs=1, space="PSUM") as g_psum,
            tc.tile_pool(name="acps", bufs=1, space="PSUM") as acc_psum,
            tc.tile_pool(name="csps", bufs=1, space="PSUM") as cs_psum,
            tc.tile_pool(name="bcps", bufs=2, space="PSUM") as bc_psum,
        ):
            # ---- constants ----
            lhsT_aug = const_pool.tile([T + 1, T], f32)   # rows 0..63 exp(transT), row 64 ones
            nc.sync.dma_start(lhsT_aug[0:T, :], transT)
            nc.scalar.activation(lhsT_aug[0:T, :], lhsT_aug[0:T, :], AF.Exp)
            nc.vector.memset(lhsT_aug[T : T + 1, :], 1.0)

            trans_sb = const_pool.tile([T, T], f32r)
            nc.sync.dma_start(trans_sb[:], trans)

            ones_col = const_pool.tile([T, 1], f32)
            nc.vector.memset(ones_col[:], 1.0)
            ones_col_r = const_pool.tile([T, 1], f32r)
            nc.scalar.activation(ones_col_r[:], ones_col[:], AF.Copy)
            ones_row = const_pool.tile([1, T], f32)
            nc.vector.memset(ones_row[:], 1.0)

            L_acc = const_pool.tile([1, BL], f32)
            nc.vector.memset(L_acc[:], 0.0)

            e_psum = acc_psum.tile([1, 512], f32)   # whole-kernel PSUM accumulators
            t_psum = acc_psum.tile([1, 512], f32)

            NSUB = K * BL // 512                    # 512-wide sub-slices per chunk

            vt_prev = None
            bc_tile = None
            first_acc = True
            for c in range(NCH):
                fr = fr_pool.tile([T, K * BL], f32)
                nc.sync.dma_start(fr[:], featsR[:, c * K : (c + 1) * K, :])
                d = d_pool.tile([T, K * BL], f32)
                nc.scalar.activation(d[:], fr[:], AF.Exp)
                ohm8 = ohm8_pool.tile([T, (K + 1) * BL], u8)
                nc.sync.dma_start(ohm8[:], ohmpad[:, c * K : c * K + K + 1, :])
                ohmf = ohmf_pool.tile([T, (K + 1) * BL], f32r)
                nc.scalar.activation(ohmf[:], ohm8[:], AF.Copy)

                vt = v_pool.tile([T + 1, (K + 1) * BL], f32)
                if c == 0:
                    nc.vector.memset(vt[0:T, 0:BL], 0.0)
                    nc.sync.dma_start(vt[T : T + 1, 0 : (K + 1) * BL], dmrow[0 : K + 1, :])
                else:
                    nc.vector.tensor_copy(vt[:, 0:BL], vt_prev[:, K * BL : (K + 1) * BL])
                    hi = min(c * K + K + 1, S)  # slice S needs no dm row
                    nc.sync.dma_start(
                        vt[T : T + 1, BL : (hi - c * K) * BL],
                        dmrow[c * K + 1 : hi, :],
                    )

                # ---- the serial chain ----
                for l in range(1, K + 1):
                    s = c * K + l
                    ps = chain_psum.tile([T, BL], f32)
                    nc.tensor.matmul(
                        ps[:],
                        lhsT_aug[:],
                        vt[0 : T + 1, (l - 1) * BL : l * BL],
                        start=True,
                        stop=True,
                    )
                    nc.vector.tensor_mul(
                        vt[0:T, l * BL : (l + 1) * BL],
                        ps[:],
                        d[:, (l - 1) * BL : l * BL],
                    )
                    if s % RESC == RESC - STALE and s + STALE <= S:
                        # stale column-sum for the rescale two steps ahead
                        cs = cs_psum.tile([1, BL], f32, tag="cs")
                        nc.tensor.matmul(
                            cs[:],
                            ones_col[:],
                            vt[0:T, l * BL : (l + 1) * BL],
                            start=True,
                            stop=True,
                        )
                        mx = small_pool.tile([1, BL], f32, tag="mx")
                        nc.vector.tensor_scalar_max(mx[:], cs[:], 1.0)
                        rc = small_pool.tile([1, BL], f32, tag="rc")
                        nc.vector.reciprocal(rc[:], mx[:])
                        bc_tile = bc_psum.tile([T, BL], f32)
                        nc.tensor.matmul(bc_tile[:], ones_row[:], rc[:], start=True, stop=True)
                        lg = small_pool.tile([1, BL], f32, tag="lg")
                        nc.scalar.activation(lg[:], mx[:], AF.Ln)
                        nc.vector.tensor_add(L_acc[:], L_acc[:], lg[:])
                    if s % RESC == 0 and bc_tile is not None:
                        nc.vector.tensor_mul(
                            vt[0:T, l * BL : (l + 1) * BL],
                            vt[0:T, l * BL : (l + 1) * BL],
                            bc_tile[:],
                        )

                # ---- gold-score streams (overlap with the chain) ----
                ep = prod_pool.tile([T, K * BL], f32r, tag="ep")
                tp = prod_pool.tile([T, K * BL], f32r, tag="tp")
                for q in range(NSUB):
                    a = q * 512
                    nc.vector.tensor_mul(
                        ep[:, a : a + 512],
                        fr[:, a : a + 512],
                        ohmf[:, BL + a : BL + a + 512],
                    )
                    gp = g_psum.tile([T, 512], f32)
                    nc.tensor.matmul(
                        gp[:],
                        trans_sb[:],
                        ohmf[:, BL + a : BL + a + 512],
                        start=True,
                        stop=True,
                    )
                    nc.vector.tensor_mul(tp[:, a : a + 512], gp[:], ohmf[:, a : a + 512])
                for q in range(NSUB):
                    a = q * 512
                    nc.tensor.matmul(
                        e_psum[:],
                        ones_col_r[:],
                        ep[:, a : a + 512],
                        start=first_acc,
                        stop=(c == NCH - 1 and q == NSUB - 1),
                        skip_group_check=True,
                    )
                    nc.tensor.matmul(
                        t_psum[:],
                        ones_col_r[:],
                        tp[:, a : a + 512],
                        start=first_acc,
                        stop=(c == NCH - 1 and q == NSUB - 1),
                        skip_group_check=True,
                    )
                    first_acc = False
                vt_prev = vt

            # ---- finalization ----
            zc = cs_psum.tile([1, BL], f32, tag="cs")
            nc.tensor.matmul(
                zc[:], ones_col[:], vt_prev[0:T, K * BL : (K + 1) * BL], start=True, stop=True
            )
            lz = small_pool.tile([1, BL], f32, tag="lz")
            nc.scalar.activation(lz[:], zc[:], AF.Ln)
            fwd = small_pool.tile([1, BL], f32, tag="fwd")
            nc.vector.tensor_add(fwd[:], lz[:], L_acc[:])

            e_sum = small_pool.tile([1, BL], f32, tag="esum")
            nc.vector.tensor_reduce(
                e_sum[:],
                e_psum[:].rearrange("p (q b) -> p b q", b=BL),
                axis=AX.X,
                op=ALU.add,
            )
            t_sum = small_pool.tile([1, BL], f32, tag="tsum")
            nc.vector.tensor_reduce(
                t_sum[:],
                t_psum[:].rearrange("p (q b) -> p b q", b=BL),
                axis=AX.X,
                op=ALU.add,
            )
            gold = small_pool.tile([1, BL], f32, tag="gold")
            nc.vector.tensor_add(gold[:], e_sum[:], t_sum[:])
            res = small_pool.tile([1, BL], f32, tag="res")
            nc.vector.tensor_sub(res[:], fwd[:], gold[:])
            nc.sync.dma_start(out_parts, res[:])

    return nc


def _get_program():
    global _PROGRAM
    if _PROGRAM is None:
        _PROGRAM = _build_program()
    return _PROGRAM


def _prepare_inputs(feats, transitions, start_transitions, end_transitions, tags, mask):
    feats = np.asarray(feats, np.float32)
    transitions = np.asarray(transitions, np.float32)
    start_transitions = np.asarray(start_transitions, np.float32)
    end_transitions = np.asarray(end_transitions, np.float32)
    tags = np.asarray(tags)
    mask = np.asarray(mask, bool)

    lengths = mask.sum(1).astype(np.int64)          # (B,)
    feats2 = feats.copy()
    feats2[:, 0, :] += start_transitions[None, :]
    feats2[np.arange(B), lengths - 1, :] += end_transitions[None, :]

    maskf = mask.astype(np.float32)
    dm = maskf - np.concatenate([maskf[:, 1:], np.zeros((B, 1), np.float32)], 1)
    # reversed time: row s <-> t = S-1-s
    dmrow_full = np.ascontiguousarray(dm[:, ::-1].T)              # (S, B)

    oh = (tags[:, :, None] == np.arange(T)[None, None, :]) & mask[:, :, None]
    ohmR = oh[:, ::-1, :].transpose(2, 1, 0).astype(np.uint8)     # (T, S, B)
    ohmpad_full = np.zeros((T, S + 1, B), np.uint8)
    ohmpad_full[:, 1:, :] = ohmR

    featsR_full = feats2[:, ::-1, :].transpose(2, 1, 0)           # (T, S, B)

    transT = np.ascontiguousarray(transitions.T)

    in_maps = []
    for c in range(NCORES):
        sl = slice(c * BL, (c + 1) * BL)
        in_maps.append(
            {
                "featsR": np.ascontiguousarray(featsR_full[:, :, sl]),
                "ohmpad": np.ascontiguousarray(ohmpad_full[:, :, sl]),
                "dmrow": np.ascontiguousarray(dmrow_full[:, sl]),
                "transT": transT,
                "trans": transitions,
            }
        )
    return in_maps


_MAXW = 1
_PATCHED = False


def _legalize_bir(bir_json: bytes) -> bytes:
    """Split >_MAXW sync waits per instruction onto preceding NoOps.

    The walrus build in this container rejects instructions carrying
    more sync waits than its TPB_CTRL/LW encodings allow ("Too many
    sync wait commands"). A NoOp on the same engine directly before the
    instruction waits on the excess semaphores first — same semantics,
    legal encoding.
    """
    import json

    m = json.loads(bir_json)
    ctr = [0]
    for f in m["functions"]:
        for bb in f["blocks"]:
            insts = bb.get("instructions") or []
            out = []
            changed = False
            for ins in insts:
                si = ins.get("sync_info")
                waits = (si or {}).get("on_wait") or []
                if len(waits) > _MAXW:
                    changed = True
                    extra, keep = waits[: -_MAXW], waits[-_MAXW:]
                    for i in range(0, len(extra), _MAXW):
                        ctr[0] += 1
                        out.append(
                            {
                                "debug": 13,
                                "engine": ins["engine"],
                                "ins": [],
                                "name": f"I-waitfix-{ctr[0]}",
                                "opcode": "NoOp",
                                "outs": [],
                                "sync_info": {
                                    "on_update": [],
                                    "on_wait": extra[i : i + _MAXW],
                                },
                                "text_hint": "wait_split",
                            }
                        )
                    si["on_wait"] = keep
                out.append(ins)
            if changed:
                bb["instructions"] = out
    return json.dumps(m).encode()


def _install_compile_patch():
    global _PATCHED
    if _PATCHED:
        return
    import concourse.bass_utils as bu
    import concourse.bass2jax as b2j

    orig = bu.compile_bir_kernel

    def patched(bir_json, tmpdir, neff_name="file.neff"):
        return orig(_legalize_bir(bir_json), tmpdir, neff_name)

    bu.compile_bir_kernel = patched
    b2j.compile_bir_kernel = patched
    _PATCHED = True


def _run_sharded(nc, in_maps, bench_iters=0):
    """run_bass_via_pjrt, plus optional steady-state timing with inputs
    resident on device (no NTFF hook in this container, so the best HW
    proxy is min wall time of the jitted NEFF execution)."""
    import time

    import jax
    import concourse.mybir as mybir
    from concourse import bass2jax as b2
    from jax.sharding import Mesh, NamedSharding, PartitionSpec
    from jax.experimental.shard_map import shard_map

    b2.install_neuronx_cc_hook()

    partition_name = nc.partition_id_tensor.name if nc.partition_id_tensor else None
    in_names, out_names, out_avals, zero_outs = [], [], [], []
    for alloc in nc.m.functions[0].allocations:
        if not isinstance(alloc, mybir.MemoryLocationSet):
            continue
        name = alloc.memorylocations[0].name
        if alloc.kind == "ExternalInput":
            if name != partition_name:
                in_names.append(name)
        elif alloc.kind == "ExternalOutput":
            dtype = mybir.dt.np(alloc.dtype)
            out_names.append(name)
            out_avals.append(jax.core.ShapedArray(tuple(alloc.tensor_shape), dtype))
            zero_outs.append(np.zeros(alloc.tensor_shape, dtype))

    n_params = len(in_names)
    n_outs = len(out_names)
    in_names = in_names + out_names
    if partition_name is not None:
        in_names.append(partition_name)

    def _body(*args):
        operands = list(args)
        if partition_name is not None:
            operands.append(b2.partition_id_tensor())
        outs = b2._bass_exec_p.bind(
            *operands,
            out_avals=tuple(out_avals),
            in_names=tuple(in_names),
            out_names=tuple(out_names),
            lowering_input_output_aliases=(),
            sim_require_finite=True,
            sim_require_nnan=True,
            nc=nc,
        )
        return tuple(outs)

    devices = jax.devices()[:NCORES]
    mesh = Mesh(np.asarray(devices), ("core",))
    spec = PartitionSpec("core")
    sharded = jax.jit(
        shard_map(
            _body,
            mesh=mesh,
            in_specs=(spec,) * (n_params + n_outs),
            out_specs=(spec,) * n_outs,
            check_rep=False,
        ),
        donate_argnums=tuple(range(n_params, n_params + n_outs)),
        keep_unused=True,
    )
    concat_in = [
        np.concatenate([np.asarray(in_maps[c][in_names[i]]) for c in range(NCORES)], 0)
        for i in range(n_params)
    ]
    concat_zeros = [
        np.zeros((NCORES * z.shape[0], *z.shape[1:]), z.dtype) for z in zero_outs
    ]
    sh = NamedSharding(mesh, spec)
    dev_in = [jax.device_put(a, sh) for a in concat_in]
    out_arrs = jax.block_until_ready(sharded(*dev_in, *concat_zeros))

    best_ns = None
    for _ in range(bench_iters):
        zs = [np.zeros_like(z) for z in concat_zeros]
        t0 = time.perf_counter()
        jax.block_until_ready(sharded(*dev_in, *zs))
        dt = time.perf_counter() - t0
        best_ns = dt * 1e9 if best_ns is None else min(best_ns, dt * 1e9)

    results = [
        {
            name: np.asarray(out_arrs[i]).reshape(NCORES, *out_avals[i].shape)[c]
            for i, name in enumerate(out_names)
        }
        for c in range(NCORES)
    ]
    return results, best_ns


def run(inputs, bench_iters=0):
    """Run on 8 cores; returns (loss_scalar, exec_time_ns or None)."""
    _install_compile_patch()
    nc = _get_program()
    in_maps = _prepare_inputs(**inputs)
    results, best_ns = _run_sharded(nc, in_maps, bench_iters=bench_iters)
    parts = np.concatenate([m["out_parts"][0] for m in results])  # (256,)
    loss = np.float32(np.sum(parts, dtype=np.float64))
    return loss, best_ns


def kernel(feats, transitions, start_transitions, end_transitions, tags, mask):
    loss, _ = run(
        dict(
            feats=feats,
            transitions=transitions,
            start_transitions=start_transitions,
            end_transitions=end_transitions,
            tags=tags,
            mask=mask,
        )
    )
    return np.asarray(loss, np.float32)
